# revision 2
# baseline (speedup 1.0000x reference)
"""Distance transform kernel for Trainium2 (8 NeuronCores, SPMD) — v2.

Same algorithm as the baseline (separable two-phase exact distance
transform, windowed fast path + exact full-width fallback), restructured
to minimize the dependent-instruction chain and DMA latency:

- ONE input DMA on the SP HWDGE queue (lowest issue->visible latency of
  all queues: ~2.4us fixed) instead of sync-HWDGE + Pool-SWDGE split.
- union-over-batch as a 3-op DVE max tree on the full [24,1024] bf16
  tile (2x DVE mode), not 4 ops.
- d1 is squared on DVE *before* the PE transpose, so phase 2 reads the
  transposed d1^2 directly out of PSUM -- the ACT square / PSUM->SBUF
  copy disappears from the critical path.
- phase 2 is a single wide DVE add over an overlapping strided PSUM
  view + a single segmented min-reduce (bf16), not a DVE/Pool split.
- ACT does only the final sqrt (bf16 -> f32), its func table preloaded
  during the input DMA by an early dummy sqrt.

Sharding and host-side contract are identical to the baseline.
"""

import ml_dtypes
import numpy as np

import bass_rust
import concourse.bacc as bacc
import concourse.masks as masks
import concourse.mybir as mybir
import concourse.tile as tile
from concourse.bass_utils import run_bass_kernel_spmd

H = 128          # grid height == width
B = 8            # batch
NCORES = 8
TI = H // NCORES  # output rows per core
HR = 24          # halo rows per core (windowed program)
WIN = 8          # phase-2 h-window per output row
DMAX = 3.0       # windowed result exact iff max distance <= DMAX

DT = mybir.dt.float32
BF = mybir.dt.bfloat16
SENTINEL = 1.0e4   # penalty for non-boundary pixels (>> max real distance)
SCAN_INIT = 1.0e9  # initial scan state

_CACHE: dict = {}


# ---------------------------------------------------------------- windowed --

def _dmas_win(nc, pool, fm_d):
    """Single input DMA on the SP HWDGE queue (fastest fixed path)."""
    fm3 = fm_d.rearrange("b c h w -> h (b c) w")  # [HR, B, H]
    fm = pool.tile([HR, B, H], fm_d.dtype, tag="fm")
    nc.sync.dma_start(fm[:], fm3[:])
    return fm


def _body_win(nc, tc, pool, psumpool, fm_d, out_d, smat, wq, ones2,
              dmas=None):
    Alu = mybir.AluOpType
    fm = dmas if dmas is not None else _dmas_win(nc, pool, fm_d)
    fdt = fm[:].dtype
    fm2 = fm[:].rearrange("p b w -> p (b w)")  # [HR, 8*H]

    # union over batch: 3-op max tree, all DVE (bf16 2x mode)
    u = pool.tile([HR, 4 * H], fdt, tag="u")
    nc.vector.tensor_tensor(u[:], fm2[:, 0:4 * H], fm2[:, 4 * H:8 * H],
                            op=Alu.max)
    v = pool.tile([HR, 2 * H], fdt, tag="v")
    nc.vector.tensor_tensor(v[:], u[:, 0:2 * H], u[:, 2 * H:4 * H],
                            op=Alu.max)
    mx = pool.tile([HR, H], fdt, tag="mx")
    nc.vector.tensor_tensor(mx[:], v[:, 0:H], v[:, H:2 * H], op=Alu.max)

    # penalty: 0 where boundary (trunc16(v) >= 0.5 <=> v > 0.5), else SENT.
    # Immediate scalars keep this a non-Ptr op (no SEQ drain before it).
    pen = pool.tile([HR, H], fdt, tag="pen")
    nc.vector.tensor_scalar(out=pen[:], in0=mx[:], scalar1=0.5,
                            scalar2=SENTINEL, op0=Alu.is_lt, op1=Alu.mult)

    # phase 1: 1D distance per row via two hardware scans (fwd + reversed)
    fsc = pool.tile([HR, H], fdt, tag="fsc")
    nc.vector.tensor_tensor_scan(fsc[:], ones2[0:HR, :], pen[:],
                                 SCAN_INIT, op0=Alu.add, op1=Alu.min)
    bsc = pool.tile([HR, H], fdt, tag="bsc")
    nc.vector.tensor_tensor_scan(bsc[:], ones2[0:HR, :], pen[:, ::-1],
                                 SCAN_INIT, op0=Alu.add, op1=Alu.min)
    d1 = pool.tile([HR, H], fdt, tag="d1")
    nc.vector.tensor_tensor(d1[:], fsc[:], bsc[:, ::-1], op=Alu.min)
    # d1^2 into rows 0:HR of the matmul weights tile (rows HR:32 are
    # const 0, row 32 is const 1.0)
    nc.vector.tensor_tensor(wq[0:HR, :], d1[:], d1[:], op=Alu.mult)

    # phase 2 via ONE PE matmul: out[j, m] = sum_h wq[h, j] * smat[h, m]
    #   = d1^2[il+k, j] + (k-4)^2   for m = il*WIN + k
    # (smat rows 0:HR are the 0/1 window-selection matrix, rows HR:32 are
    # zero, row 32 is the (k-4)^2 parabola row matched to wq's ones row).
    # This transposes, expands the 16 overlapping windows, and adds the
    # parabola in one instruction.
    big = psumpool.tile([H, TI * WIN], DT, tag="big")
    nc.tensor.matmul(big[:], wq[0:33, :], smat[0:33, :],
                     start=True, stop=True)

    # sqrt each candidate straight out of PSUM (min of sqrt = sqrt of min),
    # then one segmented min-reduce produces the output block.
    rsq = pool.tile([H, TI * WIN], DT, tag="rsq")
    nc.scalar.sqrt(rsq[:], big[:])
    res = pool.tile([H, TI], DT, tag="res")
    nc.vector.tensor_reduce(res[:],
                            rsq[:].rearrange("p (i k) -> p i k", k=WIN),
                            axis=mybir.AxisListType.X, op=Alu.min)
    nc.sync.dma_start(out_d, res[:])


def _consts_win(nc, pool):
    """On-device constants, built in the input-DMA shadow."""
    Alu = mybir.AluOpType
    # ones: scan increment rows; rows 0:33 also feed affine_select
    ones2 = pool.tile([33, H], BF, tag="ones2")
    nc.gpsimd.memset(ones2[:], 1.0)
    # dummy early sqrt preloads the ACT func table during the input DMA
    dum = pool.tile([H, 1], DT, tag="dum")
    nc.gpsimd.memset(dum[:], 1.0)
    dums = pool.tile([H, 1], DT, tag="dums")
    nc.scalar.sqrt(dums[:], dum[:])
    # selection matrix smat[h, il*WIN+k] = (h == il+k) for h < HR; rows
    # HR:32 come out zero (h > il+k there), row 32 gets the parabola
    smat = pool.tile([64, TI * WIN], BF, tag="smat")
    nc.gpsimd.affine_select(smat[0:33, :], ones2[0:33, :],
                            pattern=[[1, TI], [1, WIN]],
                            compare_op=Alu.is_equal, fill=0.0,
                            base=0, channel_multiplier=-1)
    # parabola row built in place (same-base accesses only): iota the
    # k-4 ramp into row 32 (exact in bf16 for |v| <= 4), then square it
    nc.gpsimd.iota(smat[32:33, :], pattern=[[0, TI], [1, WIN]],
                   base=-WIN // 2, channel_multiplier=0,
                   allow_small_or_imprecise_dtypes=True)
    nc.gpsimd.tensor_tensor(smat[32:33, :], smat[32:33, :], smat[32:33, :],
                            op=Alu.mult)
    # matmul weights tile: rows 0:HR written per-body with d1^2, rows
    # HR:32 are zero (paired with zero smat rows), row 32 is the
    # constant 1.0 that multiplies the parabola row of smat
    wq = pool.tile([64, H], BF, tag="wq")
    nc.gpsimd.memset(wq[0:64, :], 0.0)
    nc.gpsimd.memset(wq[32:64, :], 1.0)
    return smat, wq, ones2


def _build_win(repeat: int = 1, hw_loop_iters: int = 0):
    nc = bacc.Bacc("TRN2", target_bir_lowering=False, debug=False,
                   num_devices=NCORES)
    fm_d = nc.dram_tensor("fm", [B, 1, HR, H], BF, kind="ExternalInput").ap()
    out_d = nc.dram_tensor("out", [H, TI], DT, kind="ExternalOutput").ap()

    with tile.TileContext(nc) as tc:
        with tc.tile_pool(name="main", bufs=1) as pool, \
             tc.tile_pool(name="psum", bufs=1, space="PSUM") as psumpool:
            dmas = None
            if not hw_loop_iters and repeat == 1:
                dmas = _dmas_win(nc, pool, fm_d)
            smat, wq, ones2 = _consts_win(nc, pool)
            if hw_loop_iters:
                with tc.For_i(0, hw_loop_iters, 1):
                    _body_win(nc, tc, pool, psumpool, fm_d, out_d,
                              smat, wq, ones2)
            else:
                for _rep in range(repeat):
                    _body_win(nc, tc, pool, psumpool, fm_d, out_d,
                              smat, wq, ones2,
                              dmas=dmas if _rep == 0 else None)
    nc.compile()
    return nc


# -------------------------------------------------------------- full (exact)

def _dmas_full(nc, pool, fm_d, ib_d):
    hb = B // 2
    fdt = fm_d.dtype
    fm3 = fm_d.rearrange("b c h w -> h (b c) w")
    fmb = pool.tile([H, hb, H], fdt, tag="fmb")
    nc.gpsimd.dma_start(fmb[:], fm3[:, hb:B])
    fma = pool.tile([H, hb, H], fdt, tag="fma")
    nc.sync.dma_start(fma[:], fm3[:, 0:hb])
    ibx = pool.tile([H, 2 * TI], DT, tag="ibx")
    nc.scalar.dma_start(ibx[:], ib_d)
    return fma, fmb, ibx


def _body_full(nc, tc, pool, psumpool, fm_d, ib_d, out_d,
               ident, iota_f, iotasq, ones, sent, dmas=None):
    Alu = mybir.AluOpType
    rows, win = H, H
    if dmas is None:
        dmas = _dmas_full(nc, pool, fm_d, ib_d)
    fma, fmb, ibx = dmas
    m2i = ibx[:, 0:TI]
    isq = ibx[:, TI:2 * TI]

    fdt = fma[:].dtype
    ma = pool.tile([rows, 2 * H], fdt, tag="ma")
    fma2 = fma[:].rearrange("p b w -> p (b w)")
    fmb2 = fmb[:].rearrange("p b w -> p (b w)")
    nc.vector.tensor_tensor(ma[:], fma2[:, 0:2 * H],
                            fma2[:, 2 * H:4 * H], op=Alu.max)
    mb = pool.tile([rows, 2 * H], fdt, tag="mb")
    nc.vector.tensor_tensor(mb[:], fmb2[:, 0:2 * H],
                            fmb2[:, 2 * H:4 * H], op=Alu.max)
    m2t = pool.tile([rows, 2 * H], fdt, tag="m2t")
    nc.vector.tensor_tensor(m2t[:], ma[:], mb[:], op=Alu.max)
    mx = pool.tile([rows, H], fdt, tag="mx")
    nc.vector.tensor_tensor(mx[:], m2t[:, 0:H], m2t[:, H:2 * H], op=Alu.max)

    pen = pool.tile([rows, H], DT, tag="pen")
    nc.vector.tensor_scalar(out=pen[:], in0=mx[:], scalar1=0.5,
                            scalar2=sent[0:rows, 0:1],
                            op0=Alu.is_le, op1=Alu.mult)

    fsc = pool.tile([rows, H], DT, tag="fsc")
    d1 = pool.tile([rows, H], DT, tag="d1")
    nc.vector.tensor_tensor_scan(fsc[:], ones[0:rows, :], pen[:],
                                 SCAN_INIT, op0=Alu.add, op1=Alu.min)
    bsc = pool.tile([rows, H], DT, tag="bscr")
    nc.vector.tensor_tensor_scan(bsc[:], ones[0:rows, :],
                                 pen[:, ::-1], SCAN_INIT,
                                 op0=Alu.add, op1=Alu.min)
    nc.vector.tensor_tensor(d1[:], fsc[:], bsc[:, ::-1], op=Alu.min)

    pt = psumpool.tile([H, rows], DT, tag="pt")
    nc.tensor.transpose(pt[:], d1[:], ident[:])
    t2 = pool.tile([H, rows], DT, tag="t2")
    nc.scalar.square(t2[:], pt[:])

    nd = 10
    np_ = TI - nd
    bigt = pool.tile([H, TI * win], DT, tag="bigt")
    biga = bigt[:, 0:nd * win]
    bigb = bigt[:, nd * win:TI * win]
    d2 = pool.tile([H, TI], DT, tag="d2")

    t2h = pool.tile([H, rows], DT, tag="t2h")
    nc.vector.tensor_tensor(t2h[:], t2[:], iotasq[:, 0:rows], op=Alu.add)
    for il in range(nd):
        nc.vector.scalar_tensor_tensor(
            out=biga[:, il * win:(il + 1) * win], in0=iota_f[:, 0:win],
            scalar=m2i[:, il:il + 1], in1=t2h[:, 0:win],
            op0=Alu.mult, op1=Alu.add)
    for il in range(nd, TI):
        k = il - nd
        sl = slice(k * win, (k + 1) * win)
        nc.gpsimd.tensor_scalar(
            out=bigb[:, sl], in0=iota_f[:, 0:win],
            scalar1=m2i[:, il:il + 1], scalar2=None, op0=Alu.mult)
        nc.gpsimd.tensor_tensor(bigb[:, sl], bigb[:, sl],
                                t2h[:, 0:win], op=Alu.add)

    nc.vector.tensor_reduce(
        d2[:, 0:nd], biga.rearrange("p (i h) -> p i h", h=win),
        axis=mybir.AxisListType.X, op=Alu.min)
    nc.vector.tensor_reduce(
        d2[:, nd:TI], bigb.rearrange("p (i h) -> p i h", h=win),
        axis=mybir.AxisListType.X, op=Alu.min)

    d2f = pool.tile([H, TI], DT, tag="d2f")
    nc.vector.tensor_tensor(d2f[:], d2[:], isq[:], op=Alu.add)
    res = pool.tile([H, TI], DT, tag="res")
    nc.scalar.sqrt(res[:], d2f[:])
    nc.sync.dma_start(out_d, res[:])


def _build_full():
    Alu = mybir.AluOpType
    nc = bacc.Bacc("TRN2", target_bir_lowering=False, debug=False,
                   num_devices=NCORES)
    fm_d = nc.dram_tensor("fm", [B, 1, H, H], DT, kind="ExternalInput").ap()
    ib_d = nc.dram_tensor("ibias", [H, 2 * TI], DT, kind="ExternalInput").ap()
    out_d = nc.dram_tensor("out", [H, TI], DT, kind="ExternalOutput").ap()

    with tile.TileContext(nc) as tc:
        with tc.tile_pool(name="main", bufs=1) as pool, \
             tc.tile_pool(name="psum", bufs=1, space="PSUM") as psumpool:
            dmas = _dmas_full(nc, pool, fm_d, ib_d)
            ident = pool.tile([H, H], DT, tag="ident")
            masks.make_identity(nc, ident[:])
            sent2 = pool.tile([H, 1], DT, tag="sent2")
            nc.gpsimd.memset(sent2[:], SENTINEL * SENTINEL)
            sent = pool.tile([H, 1], DT, tag="sent")
            nc.scalar.sqrt(sent[:], sent2[:])
            iota_i = pool.tile([H, H], mybir.dt.int32, tag="iota_i")
            nc.gpsimd.iota(iota_i[:], pattern=[[1, H]], base=0,
                           channel_multiplier=0)
            iota_f = pool.tile([H, H], DT, tag="iota_f")
            nc.vector.tensor_copy(iota_f[:], iota_i[:])
            iotasq = pool.tile([H, H], DT, tag="iotasq")
            nc.scalar.square(iotasq[:], iota_f[:])
            ones = pool.tile([H, H], DT, tag="ones")
            nc.gpsimd.memset(ones[:], 1.0)
            _body_full(nc, tc, pool, psumpool, fm_d, ib_d, out_d,
                       ident, iota_f, iotasq, ones, sent, dmas=dmas)
    nc.compile()
    return nc


# ------------------------------------------------------------------- host --

def _build_program(windowed: bool, repeat: int = 1, hw_loop_iters: int = 0):
    if windowed:
        return _build_win(repeat=repeat, hw_loop_iters=hw_loop_iters)
    assert repeat == 1 and not hw_loop_iters
    return _build_full()


def _get_program(windowed: bool):
    key = "win" if windowed else "full"
    if key not in _CACHE:
        _CACHE[key] = _build_program(windowed)
    return _CACHE[key]


def _in_maps(feature_map: np.ndarray, windowed: bool):
    maps = []
    for c in range(NCORES):
        if windowed:
            # halo rows are true h in [16c-WIN/2, ...), zero-padded outside
            # the grid. Shipped as truncated bf16: v > 0.5 <=> trunc16(v)
            # >= 0.5 for v != 0.5 (v == 0.5 exactly is host-guarded).
            lo = TI * c - WIN // 2
            fm_c = np.zeros((B, 1, HR, H), np.float32)
            s, e = max(0, lo), min(H, lo + HR)
            fm_c[:, :, s - lo:e - lo, :] = feature_map[:, :, s:e, :]
            fm_bf = (np.ascontiguousarray(fm_c).view(np.uint32) >> 16) \
                .astype(np.uint16).view(ml_dtypes.bfloat16)
            maps.append({"fm": fm_bf})
        else:
            iv = np.arange(c * TI, (c + 1) * TI, dtype=np.float32)
            row = np.concatenate([-2.0 * iv, iv * iv])
            maps.append({
                "fm": np.ascontiguousarray(feature_map),
                "ibias": np.ascontiguousarray(
                    np.broadcast_to(row[None, :], (H, 2 * TI))),
            })
    return maps


def _run(feature_map, windowed, trace=False):
    nc = _get_program(windowed)
    out = run_bass_kernel_spmd(nc, _in_maps(feature_map, windowed),
                               list(range(NCORES)), trace=trace)
    _CACHE["last_result"] = out
    # per-core block c is [128(j), 16(i_local)] with i = 16c + i_local
    cols = np.concatenate([r["out"] for r in out.results], axis=1)
    return cols.T  # [i, j]


def kernel(feature_map: np.ndarray, _trace: bool = False):
    fm = np.ascontiguousarray(np.asarray(feature_map, dtype=np.float32))
    assert fm.shape == (B, 1, H, H), fm.shape
    if np.any(fm == np.float32(0.5)):
        # bf16-truncation trick needs v != 0.5 exactly; exact full program
        dist = _run(fm, windowed=False, trace=_trace)
        return np.ascontiguousarray(
            np.broadcast_to(dist[None, None], (B, 1, H, H))
            .astype(np.float32))
    dist = _run(fm, windowed=True, trace=_trace)
    if not np.all(dist <= DMAX + 0.01):  # margin for ACT sqrt rounding
        # windowed result not provably exact -> exact full-width program
        dist = _run(fm, windowed=False, trace=_trace)
    return np.ascontiguousarray(
        np.broadcast_to(dist[None, None], (B, 1, H, H)).astype(np.float32))


# revision 3
# speedup vs baseline: 1.7362x; 1.7362x over previous
"""Distance transform kernel for Trainium2 (8 NeuronCores, SPMD) — v2.

Same algorithm as the baseline (separable two-phase exact distance
transform, windowed fast path + exact full-width fallback), restructured
to minimize the dependent-instruction chain and DMA latency:

- ONE input DMA on the SP HWDGE queue (lowest issue->visible latency of
  all queues) instead of a sync-HWDGE + Pool-SWDGE split.
- union-over-batch as a 3-op DVE max tree on the full [24,1024] bf16
  tile (2x DVE mode), not 4 ops.
- penalty op uses immediate scalars (non-Ptr op: the framework inserts
  no SEQ drain before it).
- phase 2 is ONE PE matmul: lhsT rows 0:24 hold d1^2 (so the matmul
  performs the transpose), the selection matrix expands the 16
  overlapping h-windows, and an extra ones-row x parabola-row pair adds
  (k-4)^2 -- replacing transpose + PSUM-copy + wide add of the baseline.
- ACT then sqrts every candidate straight out of PSUM (min of sqrt =
  sqrt of min) and a single DVE segmented min-reduce yields the output
  block; ACT's func table is preloaded during the input DMA.

Sharding and host-side contract are identical to the baseline.
"""

import ml_dtypes
import numpy as np

import bass_rust
import concourse.bacc as bacc
import concourse.masks as masks
import concourse.mybir as mybir
import concourse.tile as tile
from concourse.bass_utils import run_bass_kernel_spmd

H = 128          # grid height == width
B = 8            # batch
NCORES = 8
TI = H // NCORES  # output rows per core
HR = 24          # halo rows per core (windowed program)
WIN = 8          # phase-2 h-window per output row
DMAX = 3.0       # windowed result exact iff max distance <= DMAX

DT = mybir.dt.float32
BF = mybir.dt.bfloat16
SENTINEL = 1.0e4   # penalty for non-boundary pixels (>> max real distance)
SCAN_INIT = 1.0e9  # initial scan state

_CACHE: dict = {}


# ---------------------------------------------------------------- windowed --

import os as _os
SPLIT_DMA = _os.environ.get("K_SPLIT_DMA", "0") == "1"


def _dmas_win(nc, pool, fm_d, split=None):
    """Input DMA(s). split=False: one DMA on the SP HWDGE queue;
    split=True: halves on SP HWDGE + Pool SWDGE."""
    if split is None:
        split = SPLIT_DMA
    fm3 = fm_d.rearrange("b c h w -> h (b c) w")  # [HR, B, H]
    fm = pool.tile([HR, B, H], fm_d.dtype, tag="fm")
    if split:
        hb = B // 2
        nc.gpsimd.dma_start(fm[:, hb:B], fm3[:, hb:B])
        nc.sync.dma_start(fm[:, 0:hb], fm3[:, 0:hb])
    else:
        nc.sync.dma_start(fm[:], fm3[:])
    return fm


def _body_win(nc, tc, pool, psumpool, fm_d, out_d, smat, wq, ones2,
              dmas=None, split=None):
    Alu = mybir.AluOpType
    fm = dmas if dmas is not None else _dmas_win(nc, pool, fm_d, split=split)
    fdt = fm[:].dtype
    fm2 = fm[:].rearrange("p b w -> p (b w)")  # [HR, 8*H]

    # union over batch: 3-op max tree, all DVE (bf16 2x mode)
    u = pool.tile([HR, 4 * H], fdt, tag="u")
    nc.vector.tensor_tensor(u[:], fm2[:, 0:4 * H], fm2[:, 4 * H:8 * H],
                            op=Alu.max)
    v = pool.tile([HR, 2 * H], fdt, tag="v")
    nc.vector.tensor_tensor(v[:], u[:, 0:2 * H], u[:, 2 * H:4 * H],
                            op=Alu.max)
    mx = pool.tile([HR, H], fdt, tag="mx")
    nc.vector.tensor_tensor(mx[:], v[:, 0:H], v[:, H:2 * H], op=Alu.max)

    # penalty: 0 where boundary (trunc16(v) >= 0.5 <=> v > 0.5), else SENT.
    # Immediate scalars keep this a non-Ptr op (no SEQ drain before it).
    pen = pool.tile([HR, H], fdt, tag="pen")
    nc.vector.tensor_scalar(out=pen[:], in0=mx[:], scalar1=0.5,
                            scalar2=SENTINEL, op0=Alu.is_lt, op1=Alu.mult)

    # phase 1: 1D distance per row via two hardware scans (fwd + reversed)
    fsc = pool.tile([HR, H], fdt, tag="fsc")
    nc.vector.tensor_tensor_scan(fsc[:], ones2[0:HR, :], pen[:],
                                 SCAN_INIT, op0=Alu.add, op1=Alu.min)
    bsc = pool.tile([HR, H], fdt, tag="bsc")
    nc.vector.tensor_tensor_scan(bsc[:], ones2[0:HR, :], pen[:, ::-1],
                                 SCAN_INIT, op0=Alu.add, op1=Alu.min)
    d1 = pool.tile([HR, H], fdt, tag="d1")
    nc.vector.tensor_tensor(d1[:], fsc[:], bsc[:, ::-1], op=Alu.min)
    # d1^2 into rows 0:HR of the matmul weights tile (rows HR:32 are
    # const 0, row 32 is const 1.0)
    nc.vector.tensor_tensor(wq[0:HR, :], d1[:], d1[:], op=Alu.mult)

    # phase 2 via ONE PE matmul: out[j, m] = sum_h wq[h, j] * smat[h, m]
    #   = d1^2[il+k, j] + (k-4)^2   for m = il*WIN + k
    # (smat rows 0:HR are the 0/1 window-selection matrix, rows HR:32 are
    # zero, row 32 is the (k-4)^2 parabola row matched to wq's ones row).
    # This transposes, expands the 16 overlapping windows, and adds the
    # parabola in one instruction.
    big = psumpool.tile([H, TI * WIN], DT, tag="big")
    nc.tensor.matmul(big[:], wq[0:33, :], smat[0:33, :],
                     start=True, stop=True)

    # sqrt each candidate straight out of PSUM (min of sqrt = sqrt of min),
    # then one segmented min-reduce produces the output block.
    rsq = pool.tile([H, TI * WIN], DT, tag="rsq")
    nc.scalar.sqrt(rsq[:], big[:])
    res = pool.tile([H, TI], DT, tag="res")
    nc.vector.tensor_reduce(res[:],
                            rsq[:].rearrange("p (i k) -> p i k", k=WIN),
                            axis=mybir.AxisListType.X, op=Alu.min)
    nc.sync.dma_start(out_d, res[:])


def _consts_win(nc, pool):
    """On-device constants, built in the input-DMA shadow."""
    Alu = mybir.AluOpType
    # ones: scan increment rows; rows 0:33 also feed affine_select
    ones2 = pool.tile([33, H], BF, tag="ones2")
    nc.gpsimd.memset(ones2[:], 1.0)
    # dummy early sqrt preloads the ACT func table during the input DMA
    dum = pool.tile([H, 1], DT, tag="dum")
    nc.gpsimd.memset(dum[:], 1.0)
    dums = pool.tile([H, 1], DT, tag="dums")
    nc.scalar.sqrt(dums[:], dum[:])
    # selection matrix smat[h, il*WIN+k] = (h == il+k) for h < HR; rows
    # HR:32 come out zero (h > il+k there), row 32 gets the parabola
    smat = pool.tile([64, TI * WIN], BF, tag="smat")
    nc.gpsimd.affine_select(smat[0:33, :], ones2[0:33, :],
                            pattern=[[1, TI], [1, WIN]],
                            compare_op=Alu.is_equal, fill=0.0,
                            base=0, channel_multiplier=-1)
    # parabola row built in place (same-base accesses only): iota the
    # k-4 ramp into row 32 (exact in bf16 for |v| <= 4), then square it
    nc.gpsimd.iota(smat[32:33, :], pattern=[[0, TI], [1, WIN]],
                   base=-WIN // 2, channel_multiplier=0,
                   allow_small_or_imprecise_dtypes=True)
    nc.gpsimd.tensor_tensor(smat[32:33, :], smat[32:33, :], smat[32:33, :],
                            op=Alu.mult)
    # matmul weights tile: rows 0:HR written per-body with d1^2, rows
    # HR:32 are zero (paired with zero smat rows), row 32 is the
    # constant 1.0 that multiplies the parabola row of smat
    wq = pool.tile([64, H], BF, tag="wq")
    nc.gpsimd.memset(wq[0:64, :], 0.0)
    nc.gpsimd.memset(wq[32:64, :], 1.0)
    return smat, wq, ones2


def _build_win(repeat: int = 1, hw_loop_iters: int = 0, split=None):
    nc = bacc.Bacc("TRN2", target_bir_lowering=False, debug=False,
                   num_devices=NCORES)
    fm_d = nc.dram_tensor("fm", [B, 1, HR, H], BF, kind="ExternalInput").ap()
    out_d = nc.dram_tensor("out", [H, TI], DT, kind="ExternalOutput").ap()

    with tile.TileContext(nc) as tc:
        with tc.tile_pool(name="main", bufs=1) as pool, \
             tc.tile_pool(name="psum", bufs=1, space="PSUM") as psumpool:
            dmas = None
            if not hw_loop_iters and repeat == 1:
                dmas = _dmas_win(nc, pool, fm_d, split=split)
            smat, wq, ones2 = _consts_win(nc, pool)
            if hw_loop_iters:
                with tc.For_i(0, hw_loop_iters, 1):
                    _body_win(nc, tc, pool, psumpool, fm_d, out_d,
                              smat, wq, ones2, split=split)
            else:
                for _rep in range(repeat):
                    _body_win(nc, tc, pool, psumpool, fm_d, out_d,
                              smat, wq, ones2,
                              dmas=dmas if _rep == 0 else None)
    nc.compile()
    return nc


# -------------------------------------------------------------- full (exact)

def _dmas_full(nc, pool, fm_d, ib_d):
    hb = B // 2
    fdt = fm_d.dtype
    fm3 = fm_d.rearrange("b c h w -> h (b c) w")
    fmb = pool.tile([H, hb, H], fdt, tag="fmb")
    nc.gpsimd.dma_start(fmb[:], fm3[:, hb:B])
    fma = pool.tile([H, hb, H], fdt, tag="fma")
    nc.sync.dma_start(fma[:], fm3[:, 0:hb])
    ibx = pool.tile([H, 2 * TI], DT, tag="ibx")
    nc.scalar.dma_start(ibx[:], ib_d)
    return fma, fmb, ibx


def _body_full(nc, tc, pool, psumpool, fm_d, ib_d, out_d,
               ident, iota_f, iotasq, ones, sent, dmas=None):
    Alu = mybir.AluOpType
    rows, win = H, H
    if dmas is None:
        dmas = _dmas_full(nc, pool, fm_d, ib_d)
    fma, fmb, ibx = dmas
    m2i = ibx[:, 0:TI]
    isq = ibx[:, TI:2 * TI]

    fdt = fma[:].dtype
    ma = pool.tile([rows, 2 * H], fdt, tag="ma")
    fma2 = fma[:].rearrange("p b w -> p (b w)")
    fmb2 = fmb[:].rearrange("p b w -> p (b w)")
    nc.vector.tensor_tensor(ma[:], fma2[:, 0:2 * H],
                            fma2[:, 2 * H:4 * H], op=Alu.max)
    mb = pool.tile([rows, 2 * H], fdt, tag="mb")
    nc.vector.tensor_tensor(mb[:], fmb2[:, 0:2 * H],
                            fmb2[:, 2 * H:4 * H], op=Alu.max)
    m2t = pool.tile([rows, 2 * H], fdt, tag="m2t")
    nc.vector.tensor_tensor(m2t[:], ma[:], mb[:], op=Alu.max)
    mx = pool.tile([rows, H], fdt, tag="mx")
    nc.vector.tensor_tensor(mx[:], m2t[:, 0:H], m2t[:, H:2 * H], op=Alu.max)

    pen = pool.tile([rows, H], DT, tag="pen")
    nc.vector.tensor_scalar(out=pen[:], in0=mx[:], scalar1=0.5,
                            scalar2=sent[0:rows, 0:1],
                            op0=Alu.is_le, op1=Alu.mult)

    fsc = pool.tile([rows, H], DT, tag="fsc")
    d1 = pool.tile([rows, H], DT, tag="d1")
    nc.vector.tensor_tensor_scan(fsc[:], ones[0:rows, :], pen[:],
                                 SCAN_INIT, op0=Alu.add, op1=Alu.min)
    bsc = pool.tile([rows, H], DT, tag="bscr")
    nc.vector.tensor_tensor_scan(bsc[:], ones[0:rows, :],
                                 pen[:, ::-1], SCAN_INIT,
                                 op0=Alu.add, op1=Alu.min)
    nc.vector.tensor_tensor(d1[:], fsc[:], bsc[:, ::-1], op=Alu.min)

    pt = psumpool.tile([H, rows], DT, tag="pt")
    nc.tensor.transpose(pt[:], d1[:], ident[:])
    t2 = pool.tile([H, rows], DT, tag="t2")
    nc.scalar.square(t2[:], pt[:])

    nd = 10
    np_ = TI - nd
    bigt = pool.tile([H, TI * win], DT, tag="bigt")
    biga = bigt[:, 0:nd * win]
    bigb = bigt[:, nd * win:TI * win]
    d2 = pool.tile([H, TI], DT, tag="d2")

    t2h = pool.tile([H, rows], DT, tag="t2h")
    nc.vector.tensor_tensor(t2h[:], t2[:], iotasq[:, 0:rows], op=Alu.add)
    for il in range(nd):
        nc.vector.scalar_tensor_tensor(
            out=biga[:, il * win:(il + 1) * win], in0=iota_f[:, 0:win],
            scalar=m2i[:, il:il + 1], in1=t2h[:, 0:win],
            op0=Alu.mult, op1=Alu.add)
    for il in range(nd, TI):
        k = il - nd
        sl = slice(k * win, (k + 1) * win)
        nc.gpsimd.tensor_scalar(
            out=bigb[:, sl], in0=iota_f[:, 0:win],
            scalar1=m2i[:, il:il + 1], scalar2=None, op0=Alu.mult)
        nc.gpsimd.tensor_tensor(bigb[:, sl], bigb[:, sl],
                                t2h[:, 0:win], op=Alu.add)

    nc.vector.tensor_reduce(
        d2[:, 0:nd], biga.rearrange("p (i h) -> p i h", h=win),
        axis=mybir.AxisListType.X, op=Alu.min)
    nc.vector.tensor_reduce(
        d2[:, nd:TI], bigb.rearrange("p (i h) -> p i h", h=win),
        axis=mybir.AxisListType.X, op=Alu.min)

    d2f = pool.tile([H, TI], DT, tag="d2f")
    nc.vector.tensor_tensor(d2f[:], d2[:], isq[:], op=Alu.add)
    res = pool.tile([H, TI], DT, tag="res")
    nc.scalar.sqrt(res[:], d2f[:])
    nc.sync.dma_start(out_d, res[:])


def _build_full():
    Alu = mybir.AluOpType
    nc = bacc.Bacc("TRN2", target_bir_lowering=False, debug=False,
                   num_devices=NCORES)
    fm_d = nc.dram_tensor("fm", [B, 1, H, H], DT, kind="ExternalInput").ap()
    ib_d = nc.dram_tensor("ibias", [H, 2 * TI], DT, kind="ExternalInput").ap()
    out_d = nc.dram_tensor("out", [H, TI], DT, kind="ExternalOutput").ap()

    with tile.TileContext(nc) as tc:
        with tc.tile_pool(name="main", bufs=1) as pool, \
             tc.tile_pool(name="psum", bufs=1, space="PSUM") as psumpool:
            dmas = _dmas_full(nc, pool, fm_d, ib_d)
            ident = pool.tile([H, H], DT, tag="ident")
            masks.make_identity(nc, ident[:])
            sent2 = pool.tile([H, 1], DT, tag="sent2")
            nc.gpsimd.memset(sent2[:], SENTINEL * SENTINEL)
            sent = pool.tile([H, 1], DT, tag="sent")
            nc.scalar.sqrt(sent[:], sent2[:])
            iota_i = pool.tile([H, H], mybir.dt.int32, tag="iota_i")
            nc.gpsimd.iota(iota_i[:], pattern=[[1, H]], base=0,
                           channel_multiplier=0)
            iota_f = pool.tile([H, H], DT, tag="iota_f")
            nc.vector.tensor_copy(iota_f[:], iota_i[:])
            iotasq = pool.tile([H, H], DT, tag="iotasq")
            nc.scalar.square(iotasq[:], iota_f[:])
            ones = pool.tile([H, H], DT, tag="ones")
            nc.gpsimd.memset(ones[:], 1.0)
            _body_full(nc, tc, pool, psumpool, fm_d, ib_d, out_d,
                       ident, iota_f, iotasq, ones, sent, dmas=dmas)
    nc.compile()
    return nc


# ------------------------------------------------------------------- host --

def _build_program(windowed: bool, repeat: int = 1, hw_loop_iters: int = 0):
    if windowed:
        return _build_win(repeat=repeat, hw_loop_iters=hw_loop_iters)
    assert repeat == 1 and not hw_loop_iters
    return _build_full()


def _get_program(windowed: bool):
    key = "win" if windowed else "full"
    if key not in _CACHE:
        _CACHE[key] = _build_program(windowed)
    return _CACHE[key]


def _in_maps(feature_map: np.ndarray, windowed: bool):
    maps = []
    for c in range(NCORES):
        if windowed:
            # halo rows are true h in [16c-WIN/2, ...), zero-padded outside
            # the grid. Shipped as truncated bf16: v > 0.5 <=> trunc16(v)
            # >= 0.5 for v != 0.5 (v == 0.5 exactly is host-guarded).
            lo = TI * c - WIN // 2
            fm_c = np.zeros((B, 1, HR, H), np.float32)
            s, e = max(0, lo), min(H, lo + HR)
            fm_c[:, :, s - lo:e - lo, :] = feature_map[:, :, s:e, :]
            fm_bf = (np.ascontiguousarray(fm_c).view(np.uint32) >> 16) \
                .astype(np.uint16).view(ml_dtypes.bfloat16)
            maps.append({"fm": fm_bf})
        else:
            iv = np.arange(c * TI, (c + 1) * TI, dtype=np.float32)
            row = np.concatenate([-2.0 * iv, iv * iv])
            maps.append({
                "fm": np.ascontiguousarray(feature_map),
                "ibias": np.ascontiguousarray(
                    np.broadcast_to(row[None, :], (H, 2 * TI))),
            })
    return maps


def _run(feature_map, windowed, trace=False):
    nc = _get_program(windowed)
    out = run_bass_kernel_spmd(nc, _in_maps(feature_map, windowed),
                               list(range(NCORES)), trace=trace)
    _CACHE["last_result"] = out
    # per-core block c is [128(j), 16(i_local)] with i = 16c + i_local
    cols = np.concatenate([r["out"] for r in out.results], axis=1)
    return cols.T  # [i, j]


def kernel(feature_map: np.ndarray, _trace: bool = False):
    fm = np.ascontiguousarray(np.asarray(feature_map, dtype=np.float32))
    assert fm.shape == (B, 1, H, H), fm.shape
    if np.any(fm == np.float32(0.5)):
        # bf16-truncation trick needs v != 0.5 exactly; exact full program
        dist = _run(fm, windowed=False, trace=_trace)
        return np.ascontiguousarray(
            np.broadcast_to(dist[None, None], (B, 1, H, H))
            .astype(np.float32))
    dist = _run(fm, windowed=True, trace=_trace)
    if not np.all(dist <= DMAX + 0.01):  # margin for ACT sqrt rounding
        # windowed result not provably exact -> exact full-width program
        dist = _run(fm, windowed=False, trace=_trace)
    return np.ascontiguousarray(
        np.broadcast_to(dist[None, None], (B, 1, H, H)).astype(np.float32))
